# revision 13
# baseline (speedup 1.0000x reference)
"""Multi-head attention Trainium2 kernel, 8-way sharded.

Problem: x[4,2048,1024] -> qkv proj (w_qkv [3072,1024]) -> 16-head attention
with key-padding mask -> tail proj (w_tail [1024,1024]) + b_tail.

Sharding: 8 shards = 4 batches x 2 head-groups (8 heads each). Each core
computes, for its (batch b, head-group hg):
  - q/k/v projections of x[b] for its 8 heads
  - full [2048 x 2048] masked attention per head
  - partial tail matmul y_part = attn_cat @ w_tail[:, cat_slice].T
Host unshards: out[b] = y_part[2b] + y_part[2b+1] + b_tail.  No collectives.

Layouts (per core, all weights pre-transposed on host):
  xT      [1024, 2048]  x[b].T
  wqkT    [1024, 1024]  q|k rows (128/head) of w_qkv shard, transposed
  wvT     [1024,  512]  v rows (64/head) of w_qkv shard, transposed
  wtailT  [ 512, 1024]  w_tail[:, hg*512:(hg+1)*512].T
  mask    [2048] int32
Kernel computes qT/kT per head via W @ xT, V directly as x @ Wv^T (token-major),
streams S^T = K Q^T per 128-key block, exp via ACT with the mask folded in as a
per-partition bias, accumulates attn^T (+ denominator via a ones column on V)
on PE, normalizes via PE transposes + per-token reciprocal, and finishes with
the tail matmul from the stacked normalized attn^T.
"""

import numpy as np
from contextlib import ExitStack

import concourse.bass as bass
import concourse.mybir as mybir
import concourse.tile as tile
from concourse.bass_utils import run_bass_kernel_spmd
from concourse.masks import make_identity

# ---------------------------------------------------------------------------
# walrus in this env accepts at most 2 sync waits per instruction; Tile's
# scheduler emits up to 10. Post-pass: peel excess waits onto same-engine
# NoOps inserted immediately before the offending instruction (same engine
# stream position => identical synchronization semantics).
MAX_WAITS = 1


def split_excess_waits(nc):
    for fn in nc.m.functions:
        for bb in fn.blocks:
            insts = list(bb.instructions)
            out = []
            changed = False
            for inst in insts:
                si = inst.sync_info
                waits = list(si.on_wait) if si is not None else []
                if len(waits) > MAX_WAITS:
                    extra = waits[:-MAX_WAITS]
                    for ci in range(0, len(extra), MAX_WAITS):
                        chunk = extra[ci:ci + MAX_WAITS]
                        nop = mybir.InstNoOp(
                            name=f"{inst.name}-ws{ci}", ins=[], outs=[])
                        nop.engine = inst.engine
                        nop.sync_info = mybir.SyncInfo(
                            on_wait=chunk, on_update=[])
                        out.append(nop)
                    inst.sync_info = mybir.SyncInfo(
                        on_wait=waits[-MAX_WAITS:],
                        on_update=list(si.on_update))
                    changed = True
                out.append(inst)
            if changed:
                bb.instructions = out
# ---------------------------------------------------------------------------

D_MODEL = 1024
N_HEAD = 16
D_HEAD = 64
BN, T = 4, 2048
HPC = 8                      # heads per core
CAT = HPC * D_HEAD           # 512 per-core tail contraction
NKB = T // 128               # 16 key blocks
NTB = T // 128               # 16 token blocks
QH = T // 2                  # 1024, q processed in two halves
KC = D_MODEL // 128          # 8 contraction chunks
F32 = mybir.dt.float32
I32 = mybir.dt.int32

# matmul compute dtype: float32 (exact, 4 cyc/row) or float32r (1 cyc/row)
import os as _os
MM_DT = (mybir.dt.float32 if _os.environ.get("MHA_MM_DT", "f32r") == "f32"
         else mybir.dt.float32r)


def _mm(ap):
    """View an f32 AP as the matmul compute dtype."""
    if MM_DT == F32:
        return ap
    return ap.bitcast(MM_DT)


def build_nc(split_waits=True):
    nc = bass.Bass()
    xT = nc.declare_dram_parameter("xT", [D_MODEL, T], F32, isOutput=False)
    wqkT = nc.declare_dram_parameter("wqkT", [D_MODEL, HPC * 128], F32, isOutput=False)
    wvT = nc.declare_dram_parameter("wvT", [D_MODEL, CAT], F32, isOutput=False)
    wtailT = nc.declare_dram_parameter("wtailT", [CAT, D_MODEL], F32, isOutput=False)
    mask = nc.declare_dram_parameter("mask", [T], I32, isOutput=False)
    y = nc.declare_dram_parameter("y", [T, D_MODEL], F32, isOutput=True)

    with ExitStack() as ctx:
        tc = ctx.enter_context(tile.TileContext(nc))

        # ---- long-lived pools (entered first so short-lived ones stack on top)
        const = ctx.enter_context(tc.tile_pool(name="const", bufs=1))
        qk_pool = ctx.enter_context(tc.tile_pool(name="qk", bufs=1))
        vaug_pool = ctx.enter_context(tc.tile_pool(name="vaug", bufs=1))
        num_pool = ctx.enter_context(tc.tile_pool(name="num", bufs=1))

        identity = const.tile([128, 128], F32)
        make_identity(nc, identity)

        # mask -> per-key-block additive bias: (m-1)*8e9  (0 keep, -8e9 drop)
        mask_i = const.tile([128, NKB], I32)
        nc.sync.dma_start(out=mask_i, in_=mask.rearrange("(j p) -> p j", p=128))
        maskb = const.tile([128, NKB], F32)
        nc.vector.tensor_copy(out=maskb, in_=mask_i)
        nc.vector.tensor_scalar(
            out=maskb, in0=maskb, scalar1=-1.0, scalar2=8e9,
            op0=mybir.AluOpType.add, op1=mybir.AluOpType.mult,
        )

        # persistent intermeds
        # q/k of 2 heads per tile: rows (h%2)*64..+64
        qts = [qk_pool.tile([128, T], F32, tag=f"qt{j}", name=f"qt{j}") for j in range(HPC // 2)]
        kts = [qk_pool.tile([128, T], F32, tag=f"kt{j}", name=f"kt{j}") for j in range(HPC // 2)]
        # V augmented with ones column: [tok-block][128, head, 65]
        vaugs = [vaug_pool.tile([128, HPC, D_HEAD + 1], F32, tag=f"va{t}", name=f"va{t}")
                 for t in range(NTB)]
        # stacked normalized attn^T: rows = local cat index (2 heads per tile)
        nums = [num_pool.tile([128, T], F32, tag=f"nm{j}", name=f"nm{j}") for j in range(CAT // 128)]

        # ---- phase 1: projections (xT resident, freed afterwards)
        with tc.tile_pool(name="xp", bufs=1) as xp_pool:
            xts = [xp_pool.tile([128, T], F32, tag=f"x{kc}", name=f"x{kc}") for kc in range(KC)]
            for kc in range(KC):
                nc.sync.dma_start(out=xts[kc], in_=xT[kc * 128:(kc + 1) * 128, :])

            # V projection: V[tok, cat] = x @ Wv^T ; ones column appended.
            # kc-outer with 8 live PSUM banks per tb-group so wv streams.
            with tc.tile_pool(name="wv", bufs=2) as wv_pool, \
                 tc.tile_pool(name="vps", bufs=1, space="PSUM") as vps:
                for grp in range(2):
                    vp8 = [vps.tile([128, CAT], F32, tag=f"vp{i}", name=f"vp{i}")
                           for i in range(8)]
                    for kc in range(KC):
                        wv = wv_pool.tile([128, CAT], F32, tag="wv", name="wv")
                        nc.sync.dma_start(
                            out=wv, in_=wvT[kc * 128:(kc + 1) * 128, :])
                        for i in range(8):
                            tb = grp * 8 + i
                            nc.tensor.matmul(
                                vp8[i],
                                _mm(xts[kc][:, tb * 128:(tb + 1) * 128]),
                                _mm(wv),
                                start=(kc == 0), stop=(kc == KC - 1),
                            )
                    for i in range(8):
                        tb = grp * 8 + i
                        va = vaugs[tb]
                        nc.vector.memset(va[:, :, D_HEAD:D_HEAD + 1], 1.0)
                        nc.vector.tensor_copy(
                            out=va[:, :, 0:D_HEAD],
                            in_=vp8[i].rearrange("p (h d) -> p h d", h=HPC),
                        )

            # q/k projection per head: qkT = Wqk_h @ xT  -> [128 rows, T]
            with tc.tile_pool(name="wqk", bufs=2) as wqk_pool, \
                 tc.tile_pool(name="qkps", bufs=2, space="PSUM") as qkps:
                for h in range(HPC):
                    wq = wqk_pool.tile([128, KC, 128], F32, tag="wqk", name="wq")
                    nc.sync.dma_start(
                        out=wq,
                        in_=wqkT.rearrange("(kc p) c -> p kc c", p=128)[
                            :, :, h * 128:(h + 1) * 128],
                    )
                    qkp = qkps.tile([128, T], F32, tag="qkp", name="qkp")
                    for n in range(T // 512):
                        for kc in range(KC):
                            nc.tensor.matmul(
                                qkp[:, n * 512:(n + 1) * 512],
                                _mm(wq[:, kc, :]),
                                _mm(xts[kc][:, n * 512:(n + 1) * 512]),
                                start=(kc == 0), stop=(kc == KC - 1),
                            )
                    j, r0 = h // 2, (h % 2) * 64
                    nc.vector.tensor_copy(out=qts[j][r0:r0 + 64, :], in_=qkp[0:64, :])
                    nc.vector.tensor_copy(out=kts[j][r0:r0 + 64, :], in_=qkp[64:128, :])

        # ---- phase 2: attention per head, q in two halves
        with tc.tile_pool(name="p_sb", bufs=3) as p_pool, \
             tc.tile_pool(name="av_sb", bufs=2) as avsb_pool, \
             tc.tile_pool(name="r_sb", bufs=4) as r_pool, \
             tc.tile_pool(name="at_sb", bufs=2) as at_pool, \
             tc.tile_pool(name="stps", bufs=2, space="PSUM") as stps, \
             tc.tile_pool(name="avps", bufs=1, space="PSUM") as avps, \
             tc.tile_pool(name="tps", bufs=1, space="PSUM") as tps:
            for pair in range(HPC // 2):
                j = pair
                # token-major normalized attn for the head pair, per q-half:
                # [tok-part, tok-blk, cat(2 heads x 64)]
                aps = [at_pool.tile([128, QH // 128, 128], F32,
                                    tag=f"ap{hf}", name=f"ap{hf}")
                       for hf in range(2)]
                for sub in range(2):
                    h = 2 * pair + sub
                    r0 = sub * 64
                    qt = qts[j][r0:r0 + 64, :]
                    kt = kts[j][r0:r0 + 64, :]
                    for half in range(2):
                        q0 = half * QH
                        avp = avps.tile([D_HEAD + 1, QH], F32, tag="avp",
                                        name="avp")
                        for kb in range(NKB):
                            stp = stps.tile([128, QH], F32, tag="stp", name="stp")
                            for n in range(QH // 512):
                                nc.tensor.matmul(
                                    stp[:, n * 512:(n + 1) * 512],
                                    _mm(kt[:, kb * 128:(kb + 1) * 128]),
                                    _mm(qt[:, q0 + n * 512:q0 + (n + 1) * 512]),
                                    start=True, stop=True,
                                )
                            p_sb = p_pool.tile([128, QH], F32, tag="p", name="p_sb")
                            nc.scalar.activation(
                                out=p_sb, in_=stp,
                                func=mybir.ActivationFunctionType.Exp,
                                bias=maskb[:, kb:kb + 1], scale=0.125,
                            )
                            for n in range(QH // 512):
                                nc.tensor.matmul(
                                    avp[:, n * 512:(n + 1) * 512],
                                    _mm(vaugs[kb][:, h, :]),
                                    _mm(p_sb[:, n * 512:(n + 1) * 512]),
                                    start=(kb == 0), stop=(kb == NKB - 1),
                                )
                        av_sb = avsb_pool.tile([D_HEAD + 1, QH], F32,
                                               tag="avsb", name="av_sb")
                        nc.vector.tensor_copy(out=av_sb, in_=avp)
                        # normalize per token: transpose + reciprocal of the
                        # denominator row, scale into the pair buffer
                        for tb in range(QH // 128):
                            t1 = tps.tile([128, D_HEAD + 1], F32, tag="t1",
                                          name="t1")
                            nc.tensor.transpose(
                                t1,
                                av_sb[:, tb * 128:(tb + 1) * 128],
                                identity[0:D_HEAD + 1, 0:D_HEAD + 1],
                            )
                            r_sb = r_pool.tile([128, 1], F32, tag="r", name="r_sb")
                            nc.vector.reciprocal(
                                out=r_sb, in_=t1[:, D_HEAD:D_HEAD + 1])
                            nc.vector.tensor_scalar_mul(
                                aps[half][:, tb, r0:r0 + 64],
                                t1[:, 0:D_HEAD], r_sb)
                # pair done: transpose token-major pair blocks into num rows
                for half in range(2):
                    q0 = half * QH
                    for tb in range(QH // 128):
                        t2 = tps.tile([128, 128], F32, tag="t2", name="t2")
                        nc.tensor.transpose(t2, aps[half][:, tb, :], identity)
                        nc.vector.tensor_copy(
                            out=nums[j][:, q0 + tb * 128:q0 + (tb + 1) * 128],
                            in_=t2,
                        )

        # ---- phase 3: tail matmul  y[tok, out] = attn_cat @ wtailT
        with tc.tile_pool(name="wt", bufs=1) as wt_pool, \
             tc.tile_pool(name="y_sb", bufs=3) as y_pool, \
             tc.tile_pool(name="yps", bufs=2, space="PSUM") as yps:
            wts = [wt_pool.tile([128, D_MODEL], F32, tag=f"wt{c}", name=f"wt{c}")
                   for c in range(CAT // 128)]
            for c in range(CAT // 128):
                nc.sync.dma_start(out=wts[c], in_=wtailT[c * 128:(c + 1) * 128, :])
            for tb in range(NTB):
                yp = yps.tile([128, D_MODEL], F32, tag="yp")
                for n in range(D_MODEL // 512):
                    for c in range(CAT // 128):
                        nc.tensor.matmul(
                            yp[:, n * 512:(n + 1) * 512],
                            _mm(nums[c][:, tb * 128:(tb + 1) * 128]),
                            _mm(wts[c][:, n * 512:(n + 1) * 512]),
                            start=(c == 0), stop=(c == CAT // 128 - 1),
                        )
                y_sb = y_pool.tile([128, D_MODEL], F32, tag="ys")
                nc.vector.tensor_copy(out=y_sb, in_=yp)
                nc.sync.dma_start(out=y[tb * 128:(tb + 1) * 128, :], in_=y_sb)

    if split_waits:
        split_excess_waits(nc)
    return nc


_NC_CACHE = None


def _get_nc():
    global _NC_CACHE
    if _NC_CACHE is None:
        _NC_CACHE = build_nc()
    return _NC_CACHE


def make_in_maps(x, mask, w_qkv, w_tail):
    """Shard full inputs into 8 per-core input maps."""
    x = np.asarray(x, dtype=np.float32)
    mask = np.asarray(mask, dtype=np.int32)
    w_qkv = np.asarray(w_qkv, dtype=np.float32)
    w_tail = np.asarray(w_tail, dtype=np.float32)

    w3 = w_qkv.reshape(N_HEAD, 3, D_HEAD, D_MODEL)  # [head, qkv, d, dmodel]
    in_maps = []
    for c in range(8):
        b, hg = c // 2, c % 2
        heads = range(hg * HPC, (hg + 1) * HPC)
        wqk = np.concatenate(
            [w3[h, 0:2].reshape(128, D_MODEL) for h in heads], axis=0
        )  # [1024, 1024] rows = (head-local, q|k, d)
        wv = np.concatenate([w3[h, 2] for h in heads], axis=0)  # [512, 1024]
        wt = w_tail[:, hg * CAT:(hg + 1) * CAT]  # [1024, 512]
        in_maps.append({
            "xT": np.ascontiguousarray(x[b].T),
            "wqkT": np.ascontiguousarray(wqk.T),
            "wvT": np.ascontiguousarray(wv.T),
            "wtailT": np.ascontiguousarray(wt.T),
            "mask": mask[b],
        })
    return in_maps


def kernel(x, mask, w_qkv, w_tail, b_tail):
    nc = _get_nc()
    in_maps = make_in_maps(x, mask, w_qkv, w_tail)
    res = run_bass_kernel_spmd(nc, in_maps, list(range(8))).results
    out = np.empty((BN, T, D_MODEL), dtype=np.float32)
    b_tail = np.asarray(b_tail, dtype=np.float32)
    for b in range(BN):
        out[b] = res[2 * b]["y"] + res[2 * b + 1]["y"] + b_tail
    return out
